# revision 10
# baseline (speedup 1.0000x reference)
"""Trainium2 Bass kernel for CausalStdMeanScaler.

Computes, per row (b, v) along time T:
    w      = weights * padding_mask
    cw     = cumsum(w)
    cv     = cumsum(w * data)
    means  = cv / max(cw, 1)
    sm     = shift_right(means)              # zero at t=0
    inc    = (data - sm) * (data - means) * w
    m2     = cumsum(inc)
    var    = m2 / max(cw - 1, 1)
    scale  = sqrt(var + 0.1)
    scaled = (data - means) / scale
Returns (scaled, means, scale).

Sharding: fully data-parallel across 8 NeuronCores along the batch axis
(64 batches -> 8 per core; each core handles 2048 independent rows of
length 4096). The time scan stays local; no communication.

Implementation notes:
  - Row-major layout throughout: 128 rows per partition block, time along
    the free dimension. All DMAs are contiguous 4KB-per-row stretches.
  - Cumsums use the DVE/GPSIMD tensor_tensor_scan instruction
    (state = (data0 op0 state) op1 data1) with op0=add, op1=bypass.
  - shifted means need no second division: sm is an AP-shifted view of
    means (carry across chunk boundaries via the previous tile).
  - Reciprocals use reciprocal_approx_fast (~51 ULP), inputs pre-clamped
    to >= 1 (or sqrt(0.1)) so no edge cases.
  - Work is spread across DVE / GPSIMD / ACT engines (assignment in ENG).
"""

import sys

import numpy as np

sys.path.insert(0, "/opt/trn_rl_repo")

import concourse.bacc as bacc  # noqa: E402
import concourse.bass as bass  # noqa: E402
import concourse.mybir as mybir  # noqa: E402
from concourse.bass_utils import run_bass_kernel_spmd  # noqa: E402
from concourse.tile import TileContext  # noqa: E402

B, V, T = 64, 256, 4096
N_CORES = 8
ROWS_PER_CORE = (B // N_CORES) * V  # 2048
P = 128
T_CHUNK = 1024
MINIMUM_SCALE = 0.1

F32 = mybir.dt.float32
ADD = mybir.AluOpType.add
SUB = mybir.AluOpType.subtract
MULT = mybir.AluOpType.mult
MAX = mybir.AluOpType.max
BYP = mybir.AluOpType.bypass

# Engine assignment for each full-size op ('vector' = DVE, 'gpsimd' = Pool).
ENG = {
    "w": "gpsimd",    # w = wt * mask
    "wd": "gpsimd",   # wd = w * d
    "cw": "vector",   # cw = scan(w)     (scan is DVE-only)
    "cv": "vector",   # cv = scan(wd)
    "m": "gpsimd",    # means = cv * r1
    "dm": "gpsimd",   # dm = d - means
    "dsm": "vector",  # dsm = d - shift(means)
    "p": "vector",    # p = dm * dsm
    "inc": "gpsimd",  # inc = p * w
    "m2": "vector",   # m2 = scan(inc)
    "q": "gpsimd",    # q = m2 * r3
    "scaled": "vector",  # scaled = dm * inv
}


def _emit(tc, ins, outs, rows, t, t_chunk, eng=ENG):
    nc = tc.nc
    d_dram, mask_dram, wt_dram = ins
    scaled_dram, m_dram, scale_dram = outs
    nrb = rows // P
    nch = t // t_chunk
    TC = t_chunk

    def E(op):
        return getattr(nc, eng[op])

    with tc.tile_pool(name="consts", bufs=1) as cpool:
        bias_t = cpool.tile([P, 1], F32, name="bias_t")
        nc.vector.memset(bias_t, MINIMUM_SCALE)
        _emit_body(tc, ins, outs, rows, t, t_chunk, eng, bias_t)


def _emit_body(tc, ins, outs, rows, t, t_chunk, eng, bias_t):
    nc = tc.nc
    d_dram, mask_dram, wt_dram = ins
    scaled_dram, m_dram, scale_dram = outs
    nrb = rows // P
    nch = t // t_chunk
    TC = t_chunk

    def E(op):
        return getattr(nc, eng[op])

    with tc.tile_pool(name="pool", bufs=2) as pool:
        for rb in range(nrb):
            r0 = rb * P
            prev = {}
            for c in range(nch):
                t0 = c * TC
                dsl = (slice(r0, r0 + P), slice(t0, t0 + TC))

                d_t = pool.tile([P, TC], F32, name="d_t")
                mask_t = pool.tile([P, TC], F32, name="mask_t")
                wt_t = pool.tile([P, TC], F32, name="wt_t")
                nc.sync.dma_start(out=d_t, in_=d_dram[dsl])
                nc.sync.dma_start(out=mask_t, in_=mask_dram[dsl])
                nc.sync.dma_start(out=wt_t, in_=wt_dram[dsl])

                w_t = pool.tile([P, TC], F32, name="w_t")
                E("w").tensor_tensor(w_t, wt_t, mask_t, MULT)

                wd_t = pool.tile([P, TC], F32, name="wd_t")
                E("wd").tensor_tensor(wd_t, w_t, d_t, MULT)

                cw_t = pool.tile([P, TC], F32, name="cw_t")
                init_cw = prev["cw"][:, TC - 1 : TC] if c else 0.0
                E("cw").tensor_tensor_scan(cw_t, w_t, w_t, init_cw, ADD, BYP)

                cv_t = pool.tile([P, TC], F32, name="cv_t")
                init_cv = prev["cv"][:, TC - 1 : TC] if c else 0.0
                E("cv").tensor_tensor_scan(cv_t, wd_t, wd_t, init_cv, ADD, BYP)

                dnm_t = pool.tile([P, TC], F32, name="dnm_t")
                nc.vector.tensor_scalar(
                    out=dnm_t, in0=cw_t, scalar1=1.0, scalar2=None, op0=MAX
                )
                r1_t = pool.tile([P, TC], F32, name="r1_t")
                nc.vector.reciprocal_approx_fast(out=r1_t, in_=dnm_t)

                m_t = pool.tile([P, TC], F32, name="m_t")
                E("m").tensor_tensor(m_t, cv_t, r1_t, MULT)

                dm_t = pool.tile([P, TC], F32, name="dm_t")
                E("dm").tensor_tensor(dm_t, d_t, m_t, SUB)

                dsm_t = pool.tile([P, TC], F32, name="dsm_t")
                E("dsm").tensor_tensor(
                    dsm_t[:, 1:TC], d_t[:, 1:TC], m_t[:, 0 : TC - 1], SUB
                )
                if c:
                    E("dsm").tensor_tensor(
                        dsm_t[:, 0:1], d_t[:, 0:1], prev["m"][:, TC - 1 : TC], SUB
                    )
                else:
                    nc.vector.tensor_copy(dsm_t[:, 0:1], d_t[:, 0:1])

                p_t = pool.tile([P, TC], F32, name="p_t")
                E("p").tensor_tensor(p_t, dm_t, dsm_t, MULT)

                inc_t = pool.tile([P, TC], F32, name="inc_t")
                E("inc").tensor_tensor(inc_t, p_t, w_t, MULT)

                m2_t = pool.tile([P, TC], F32, name="m2_t")
                init_m2 = prev["m2"][:, TC - 1 : TC] if c else 0.0
                E("m2").tensor_tensor_scan(m2_t, inc_t, inc_t, init_m2, ADD, BYP)

                dn3_t = pool.tile([P, TC], F32, name="dn3_t")
                nc.vector.tensor_scalar(
                    out=dn3_t, in0=cw_t, scalar1=1.0, scalar2=1.0, op0=SUB, op1=MAX
                )
                r3_t = pool.tile([P, TC], F32, name="r3_t")
                nc.vector.reciprocal_approx_fast(out=r3_t, in_=dn3_t)

                q_t = pool.tile([P, TC], F32, name="q_t")
                E("q").tensor_tensor(q_t, m2_t, r3_t, MULT)

                scale_t = pool.tile([P, TC], F32, name="scale_t")
                nc.scalar.activation(
                    scale_t, q_t, mybir.ActivationFunctionType.Sqrt,
                    bias=bias_t, scale=1.0,
                )

                inv_t = pool.tile([P, TC], F32, name="inv_t")
                nc.vector.reciprocal_approx_fast(out=inv_t, in_=scale_t)

                scaled_t = pool.tile([P, TC], F32, name="scaled_t")
                E("scaled").tensor_tensor(scaled_t, dm_t, inv_t, MULT)

                nc.sync.dma_start(out=m_dram[dsl], in_=m_t)
                nc.sync.dma_start(out=scale_dram[dsl], in_=scale_t)
                nc.sync.dma_start(out=scaled_dram[dsl], in_=scaled_t)

                prev = {"cw": cw_t, "cv": cv_t, "m2": m2_t, "m": m_t}


def build(rows=ROWS_PER_CORE, t=T, t_chunk=T_CHUNK, eng=ENG, seq_codegen=False):
    nc = bacc.Bacc(
        "TRN2", debug=False, target_bir_lowering=False,
        use_seq_codegen=bool(seq_codegen),
    )
    d = nc.dram_tensor("data", [rows, t], F32, kind="ExternalInput").ap()
    mask = nc.dram_tensor("mask", [rows, t], F32, kind="ExternalInput").ap()
    wt = nc.dram_tensor("wt", [rows, t], F32, kind="ExternalInput").ap()
    scaled = nc.dram_tensor("scaled", [rows, t], F32, kind="ExternalOutput").ap()
    means = nc.dram_tensor("means", [rows, t], F32, kind="ExternalOutput").ap()
    scale = nc.dram_tensor("scale", [rows, t], F32, kind="ExternalOutput").ap()
    with TileContext(nc) as tc:
        _emit(tc, (d, mask, wt), (scaled, means, scale), rows, t, t_chunk, eng)
    nc.compile()
    return nc

_NC_CACHE = {}


def _get_nc():
    if "nc" not in _NC_CACHE:
        _NC_CACHE["nc"] = build()
    return _NC_CACHE["nc"]


LAST_EXEC_TIME_NS = None
LAST_RESULTS = None


def _run(data, padding_mask, weights, trace=False, **kw):
    """data/padding_mask/weights: full (B, V, T) float32 arrays."""
    global LAST_EXEC_TIME_NS, LAST_RESULTS
    nc = _get_nc()
    d = np.ascontiguousarray(np.asarray(data, np.float32)).reshape(
        N_CORES, ROWS_PER_CORE, T
    )
    mk = np.ascontiguousarray(np.asarray(padding_mask, np.float32)).reshape(
        N_CORES, ROWS_PER_CORE, T
    )
    wt = np.ascontiguousarray(np.asarray(weights, np.float32)).reshape(
        N_CORES, ROWS_PER_CORE, T
    )
    in_maps = [
        {"data": d[i], "mask": mk[i], "wt": wt[i]} for i in range(N_CORES)
    ]
    res = run_bass_kernel_spmd(nc, in_maps, list(range(N_CORES)), trace=trace, **kw)
    LAST_EXEC_TIME_NS = res.exec_time_ns
    LAST_RESULTS = res
    scaled = np.concatenate([np.asarray(r["scaled"]) for r in res.results])
    means = np.concatenate([np.asarray(r["means"]) for r in res.results])
    scale = np.concatenate([np.asarray(r["scale"]) for r in res.results])
    shape = (B, V, T)
    return (
        scaled.reshape(shape),
        means.reshape(shape),
        scale.reshape(shape),
    )


def kernel(data, padding_mask, weights):
    return _run(data, padding_mask, weights, trace=False)


# revision 13
# speedup vs baseline: 1.0788x; 1.0788x over previous
"""Trainium2 Bass kernel for CausalStdMeanScaler.

Computes, per row (b, v) along time T:
    w      = weights * padding_mask
    cw     = cumsum(w)
    cv     = cumsum(w * data)
    means  = cv / max(cw, 1)
    sm     = shift_right(means)              # zero at t=0
    inc    = (data - sm) * (data - means) * w
    m2     = cumsum(inc)
    var    = m2 / max(cw - 1, 1)
    scale  = sqrt(var + 0.1)
    scaled = (data - means) / scale
Returns (scaled, means, scale).

Sharding: fully data-parallel across 8 NeuronCores along the batch axis
(64 batches -> 8 per core; each core handles 2048 independent rows of
length 4096). The time scan stays local; no communication.

Implementation notes:
  - Row-major layout: 128 rows per partition block, time chunked along
    the free dimension. All DMAs are contiguous 8KB-per-row stretches.
  - Cumsums use the DVE tensor_tensor_scan instruction
    (state = (data0 op0 state) op1 data1) with op0=add, op1=bypass.
  - shifted means need no second division: sm is an AP-shifted view of
    means (carry across chunk boundaries via the previous tile).
  - Reciprocals use reciprocal_approx_fast (~51 ULP), inputs pre-clamped
    to >= 1 (or sqrt(0.1)) so no edge cases.
  - Fast path: when padding_mask is all ones (checked on host), w ==
    weights, so the mask stream and multiply are skipped entirely.
    A general-path program is built lazily if a real mask ever shows up.
  - Work is split across DVE / GPSIMD / ACT per measured op rates.
"""

import sys

import numpy as np

sys.path.insert(0, "/opt/trn_rl_repo")

import concourse.bacc as bacc  # noqa: E402
import concourse.mybir as mybir  # noqa: E402
from concourse.bass_utils import run_bass_kernel_spmd  # noqa: E402
from concourse.tile import TileContext  # noqa: E402

B, V, T = 64, 256, 4096
N_CORES = 8
ROWS_PER_CORE = (B // N_CORES) * V  # 2048
P = 128
T_CHUNK = 2048
MINIMUM_SCALE = 0.1

F32 = mybir.dt.float32
ADD = mybir.AluOpType.add
SUB = mybir.AluOpType.subtract
MULT = mybir.AluOpType.mult
MAX = mybir.AluOpType.max
BYP = mybir.AluOpType.bypass

# Engine for each full-size op ('vector' = DVE, 'gpsimd' = Pool).
# Scans / reciprocals / tensor_scalar are DVE-only (walrus rejects them
# on Pool); the tensor_tensor load is spread DVE vs GPSIMD.
ENG = {
    "w": "gpsimd",     # general path only: w = wt * mask
    "wd": "gpsimd",    # wd = w * d
    "m": "gpsimd",     # means = cv * r1
    "dm": "gpsimd",    # dm = d - means
    "dsm": "vector",   # dsm = d - shift(means)
    "p": "vector",     # p = dm * dsm
    "inc": "gpsimd",   # inc = p * w
    "q": "gpsimd",     # q = m2 * r3
    "scaled": "gpsimd",  # scaled = dm * inv
}


def _emit(tc, ins, outs, rows, t, t_chunk, eng, with_mask):
    nc = tc.nc
    with tc.tile_pool(name="consts", bufs=1) as cpool:
        bias_t = cpool.tile([P, 1], F32, name="bias_t")
        nc.vector.memset(bias_t, MINIMUM_SCALE)
        _emit_body(tc, ins, outs, rows, t, t_chunk, eng, bias_t, with_mask)


def _emit_body(tc, ins, outs, rows, t, t_chunk, eng, bias_t, with_mask):
    nc = tc.nc
    if with_mask:
        d_dram, mask_dram, wt_dram = ins
    else:
        d_dram, wt_dram = ins
    scaled_dram, m_dram, scale_dram = outs
    nrb = rows // P
    nch = t // t_chunk
    TC = t_chunk

    def E(op):
        return getattr(nc, eng[op])

    with tc.tile_pool(name="pool", bufs=2) as pool:
        for rb in range(nrb):
            r0 = rb * P
            prev = {}
            for c in range(nch):
                t0 = c * TC
                dsl = (slice(r0, r0 + P), slice(t0, t0 + TC))

                d_t = pool.tile([P, TC], F32, name="d_t")
                wt_t = pool.tile([P, TC], F32, name="wt_t")
                nc.sync.dma_start(out=d_t, in_=d_dram[dsl])
                nc.sync.dma_start(out=wt_t, in_=wt_dram[dsl])
                if with_mask:
                    mask_t = pool.tile([P, TC], F32, name="mask_t")
                    nc.sync.dma_start(out=mask_t, in_=mask_dram[dsl])
                    w_t = pool.tile([P, TC], F32, name="w_t")
                    E("w").tensor_tensor(w_t, wt_t, mask_t, MULT)
                else:
                    w_t = wt_t

                wd_t = pool.tile([P, TC], F32, name="wd_t", bufs=1)
                E("wd").tensor_tensor(wd_t, w_t, d_t, MULT)

                cw_t = pool.tile([P, TC], F32, name="cw_t")
                init_cw = prev["cw"][:, TC - 1 : TC] if c else 0.0
                nc.vector.tensor_tensor_scan(cw_t, w_t, w_t, init_cw, ADD, BYP)

                cv_t = pool.tile([P, TC], F32, name="cv_t")
                init_cv = prev["cv"][:, TC - 1 : TC] if c else 0.0
                nc.vector.tensor_tensor_scan(cv_t, wd_t, wd_t, init_cv, ADD, BYP)

                dnm_t = pool.tile([P, TC], F32, name="dnm_t", tag="tmp_ts", bufs=1)
                nc.vector.tensor_scalar(
                    out=dnm_t, in0=cw_t, scalar1=0.0, scalar2=1.0, op0=SUB, op1=MAX
                )
                r1_t = pool.tile([P, TC], F32, name="r1_t", tag="tmp_r")
                nc.vector.reciprocal_approx_fast(out=r1_t, in_=dnm_t)

                m_t = pool.tile([P, TC], F32, name="m_t")
                E("m").tensor_tensor(m_t, cv_t, r1_t, MULT)

                dm_t = pool.tile([P, TC], F32, name="dm_t")
                E("dm").tensor_tensor(dm_t, d_t, m_t, SUB)

                dsm_t = pool.tile([P, TC], F32, name="dsm_t", bufs=1)
                E("dsm").tensor_tensor(
                    dsm_t[:, 1:TC], d_t[:, 1:TC], m_t[:, 0 : TC - 1], SUB
                )
                if c:
                    E("dsm").tensor_tensor(
                        dsm_t[:, 0:1], d_t[:, 0:1], prev["m"][:, TC - 1 : TC], SUB
                    )
                else:
                    nc.vector.tensor_copy(dsm_t[:, 0:1], d_t[:, 0:1])

                p_t = pool.tile([P, TC], F32, name="p_t", bufs=1)
                E("p").tensor_tensor(p_t, dm_t, dsm_t, MULT)

                inc_t = pool.tile([P, TC], F32, name="inc_t", bufs=1)
                E("inc").tensor_tensor(inc_t, p_t, w_t, MULT)

                m2_t = pool.tile([P, TC], F32, name="m2_t")
                init_m2 = prev["m2"][:, TC - 1 : TC] if c else 0.0
                nc.vector.tensor_tensor_scan(m2_t, inc_t, inc_t, init_m2, ADD, BYP)

                dn3_t = pool.tile([P, TC], F32, name="dn3_t", tag="tmp_ts", bufs=1)
                nc.vector.tensor_scalar(
                    out=dn3_t, in0=cw_t, scalar1=1.0, scalar2=1.0, op0=SUB, op1=MAX
                )
                r3_t = pool.tile([P, TC], F32, name="r3_t", tag="tmp_r")
                nc.vector.reciprocal_approx_fast(out=r3_t, in_=dn3_t)

                q_t = pool.tile([P, TC], F32, name="q_t", tag="tmp_ts", bufs=1)
                E("q").tensor_tensor(q_t, m2_t, r3_t, MULT)

                scale_t = pool.tile([P, TC], F32, name="scale_t")
                nc.scalar.activation(
                    scale_t, q_t, mybir.ActivationFunctionType.Sqrt,
                    bias=bias_t, scale=1.0,
                )

                inv_t = pool.tile([P, TC], F32, name="inv_t", tag="tmp_r")
                nc.vector.reciprocal_approx_fast(out=inv_t, in_=scale_t)

                scaled_t = pool.tile([P, TC], F32, name="scaled_t")
                E("scaled").tensor_tensor(scaled_t, dm_t, inv_t, MULT)

                nc.sync.dma_start(out=m_dram[dsl], in_=m_t)
                nc.sync.dma_start(out=scale_dram[dsl], in_=scale_t)
                nc.sync.dma_start(out=scaled_dram[dsl], in_=scaled_t)

                prev = {"cw": cw_t, "cv": cv_t, "m2": m2_t, "m": m_t}


def build(rows=ROWS_PER_CORE, t=T, t_chunk=T_CHUNK, eng=ENG, with_mask=False):
    nc = bacc.Bacc("TRN2", debug=False, target_bir_lowering=False)
    d = nc.dram_tensor("data", [rows, t], F32, kind="ExternalInput").ap()
    ins = [d]
    if with_mask:
        ins.append(nc.dram_tensor("mask", [rows, t], F32, kind="ExternalInput").ap())
    ins.append(nc.dram_tensor("wt", [rows, t], F32, kind="ExternalInput").ap())
    scaled = nc.dram_tensor("scaled", [rows, t], F32, kind="ExternalOutput").ap()
    means = nc.dram_tensor("means", [rows, t], F32, kind="ExternalOutput").ap()
    scale = nc.dram_tensor("scale", [rows, t], F32, kind="ExternalOutput").ap()
    with TileContext(nc) as tc:
        _emit(tc, tuple(ins), (scaled, means, scale), rows, t, t_chunk, eng,
              with_mask)
    nc.compile()
    return nc


_NC_CACHE = {}


def _get_nc(with_mask):
    key = "mask" if with_mask else "ones"
    if key not in _NC_CACHE:
        # the mask variant holds 3 extra tiles; smaller chunks to fit SBUF
        tc_ = 1024 if with_mask else T_CHUNK
        _NC_CACHE[key] = build(with_mask=with_mask, t_chunk=tc_)
    return _NC_CACHE[key]


LAST_EXEC_TIME_NS = None
LAST_RESULTS = None


def _run(data, padding_mask, weights, trace=False, **kw):
    """data/padding_mask/weights: full (B, V, T) float32 arrays."""
    global LAST_EXEC_TIME_NS, LAST_RESULTS
    d = np.ascontiguousarray(np.asarray(data, np.float32)).reshape(
        N_CORES, ROWS_PER_CORE, T
    )
    mk = np.ascontiguousarray(np.asarray(padding_mask, np.float32)).reshape(
        N_CORES, ROWS_PER_CORE, T
    )
    wt = np.ascontiguousarray(np.asarray(weights, np.float32)).reshape(
        N_CORES, ROWS_PER_CORE, T
    )
    with_mask = not bool(np.all(mk == 1.0))
    nc = _get_nc(with_mask)
    if with_mask:
        in_maps = [
            {"data": d[i], "mask": mk[i], "wt": wt[i]} for i in range(N_CORES)
        ]
    else:
        in_maps = [{"data": d[i], "wt": wt[i]} for i in range(N_CORES)]
    res = run_bass_kernel_spmd(nc, in_maps, list(range(N_CORES)), trace=trace, **kw)
    LAST_EXEC_TIME_NS = res.exec_time_ns
    LAST_RESULTS = res
    scaled = np.concatenate([np.asarray(r["scaled"]) for r in res.results])
    means = np.concatenate([np.asarray(r["means"]) for r in res.results])
    scale = np.concatenate([np.asarray(r["scale"]) for r in res.results])
    shape = (B, V, T)
    return (
        scaled.reshape(shape),
        means.reshape(shape),
        scale.reshape(shape),
    )


def kernel(data, padding_mask, weights):
    return _run(data, padding_mask, weights, trace=False)
